# revision 4
# baseline (speedup 1.0000x reference)
"""Trainium2 Bass kernel for nn_CRF_13091060318991.

CRF NLL loss: out[b] = logsumexp_all_paths - gold_path_score, B=2048, S=2048,
5 BIOES emission labels (+START/END), lengths-masked forward algorithm.

Key algebraic reduction: the BIOES transition matrix from make_transition()
has rank-2 structure over the emission states (rows {O,E,S} identical, rows
{B,I} identical), so the 7-state forward recursion collapses exactly to a
2-state log-semiring recursion.  Each step is a 2x2 log-matrix; the product
over time is computed with a fully-parallel binary reduction tree (log-matmul
is associative).  Length masking = identity matrices, folded in as
plane*mask arithmetic.  Gold score is elementwise masks + fused reductions.

Sharding: pure data parallel over batch: 8 cores x 256 rows; each core runs
2 partition-tiles of 128 sequences.
"""

import numpy as np

INF = 10000.0
B, S, NL = 2048, 2048, 5
NCORES = 8
BC = B // NCORES          # 256 rows per core
P = 128                   # SBUF partitions
NT = BC // P              # 2 tiles per core
TC = 512                  # time-chunk length
NCH = S // TC             # 4 chunks
CH_LV = 3                 # tree levels inside a chunk: 512 -> 64 matrices
ACC_PER_CH = TC >> CH_LV  # 64 matrices per chunk
ACC_M = ACC_PER_CH * NCH  # 256 matrices entering the global tree
GL_LV = 8                 # 256 -> 1

_CACHE = {}


def _expected_transition():
    labs = list('BIOES')
    t = np.ones((7, 7), dtype=np.float32)
    t[:, 5] = -INF
    t[6, :] = -INF
    for i, lf in enumerate(labs):
        for j, lt in enumerate(labs):
            allowed = (lf in 'OES' and lt in 'OBS') or (lf in 'BI' and lt in 'IE')
            if not allowed:
                t[i, j] = -INF
    return t


def _build():
    import concourse.bacc as bacc
    import concourse.bass as bass
    import concourse.tile as tile
    import concourse.mybir as mybir

    F32 = mybir.dt.float32
    I32 = mybir.dt.int32
    AF = mybir.ActivationFunctionType
    OP = mybir.AluOpType
    AX = mybir.AxisListType

    nc = bacc.Bacc("TRN2", target_bir_lowering=False)
    lg_d = nc.dram_tensor("logits", [BC, S, NL], F32, kind="ExternalInput")
    lab_d = nc.dram_tensor("labels", [BC, S], I32, kind="ExternalInput")
    len_d = nc.dram_tensor("lens", [BC], I32, kind="ExternalInput")
    out_d = nc.dram_tensor("out", [BC], F32, kind="ExternalOutput")

    def sub_ap(src, extra_off, pattern):
        return bass.AP(tensor=src.tensor, offset=src.offset + extra_off,
                       ap=[src.ap[0]] + pattern)

    with tile.TileContext(nc) as tc:
        with (
            tc.tile_pool(name="singles", bufs=1) as singles,
            tc.tile_pool(name="io", bufs=2) as io,
            tc.tile_pool(name="labp", bufs=2) as labp,
            tc.tile_pool(name="nmp", bufs=2) as nmp,
            tc.tile_pool(name="tb", bufs=2) as tb,
            tc.tile_pool(name="tmp", bufs=14) as tmp,
            tc.tile_pool(name="lvl", bufs=4) as lvl,
        ):
            # time iota 0..S-1, replicated on every partition (f32)
            ii = singles.tile([P, S], I32)
            nc.gpsimd.iota(out=ii, pattern=[[1, S]], base=0, channel_multiplier=0)
            iof = singles.tile([P, S], F32)
            nc.gpsimd.tensor_copy(out=iof, in_=ii)

            def combine(src, npairs, dst):
                # src: contiguous [P, 2*npairs, 4] log-matrices (time order,
                # row-major [00,01,10,11]); dst: [P, npairs*4].
                # C_ij = LSE(A_i0 + B_0j, A_i1 + B_1j), A earlier, B later.
                W = npairs * 4
                grp = [[4, npairs], [2, 2], [1, 2]]
                a0 = sub_ap(src, 0, [[8, npairs], [2, 2], [0, 2]])
                a1 = sub_ap(src, 1, [[8, npairs], [2, 2], [0, 2]])
                b0 = sub_ap(src, 4, [[8, npairs], [0, 2], [1, 2]])
                b1 = sub_ap(src, 6, [[8, npairs], [0, 2], [1, 2]])
                t1 = tmp.tile([P, W], F32, tag="tmp")
                t2 = tmp.tile([P, W], F32, tag="tmp")
                nc.vector.tensor_tensor(out=sub_ap(t1, 0, grp), in0=a0, in1=b0, op=OP.add)
                nc.vector.tensor_tensor(out=sub_ap(t2, 0, grp), in0=a1, in1=b1, op=OP.add)
                d = tmp.tile([P, W], F32, tag="tmp")
                nc.vector.tensor_sub(out=d, in0=t1, in1=t2)
                mx = tmp.tile([P, W], F32, tag="tmp")
                nc.vector.tensor_tensor(out=mx, in0=t1, in1=t2, op=OP.max)
                ab = tmp.tile([P, W], F32, tag="tmp")
                nc.scalar.activation(out=ab, in_=d, func=AF.Abs)
                e = tmp.tile([P, W], F32, tag="tmp")
                nc.scalar.activation(out=e, in_=ab, func=AF.Exp, scale=-1.0)
                lc = tmp.tile([P, W], F32, tag="tmp")
                nc.scalar.activation(out=lc, in_=e, func=AF.Ln, bias=1.0)
                nc.vector.tensor_add(out=dst, in0=mx, in1=lc)

            for it in range(NT):
                r0 = it * P
                # ---- per-tile setup ----
                li = tb.tile([P, 1], I32, tag="li")
                nc.sync.dma_start(out=li, in_=bass.AP(tensor=len_d, offset=r0,
                                                      ap=[[1, P], [1, 1]]))
                lf = tb.tile([P, 1], F32, tag="lf")
                nc.vector.tensor_copy(out=lf, in_=li)
                lenp1 = tb.tile([P, 1], F32, tag="lenp1")
                nc.vector.tensor_scalar_add(out=lenp1, in0=lf, scalar1=1.0)
                m_t = tb.tile([P, S], F32, tag="m")          # 1.0 where t < len
                nc.vector.tensor_scalar(out=m_t, in0=iof, scalar1=lf, scalar2=None,
                                        op0=OP.is_lt)
                moff = tb.tile([P, S], F32, tag="moff")      # (m-1)*INF
                nc.vector.tensor_scalar(out=moff, in0=m_t, scalar1=-1.0, scalar2=INF,
                                        op0=OP.add, op1=OP.mult)
                accst = tb.tile([P, NCH * 5], F32, tag="accst")
                acctr = tb.tile([P, NCH], F32, tag="acctr")
                acc = tb.tile([P, ACC_M * 4], F32, tag="acc")

                for c in range(NCH):
                    # ---- loads ----
                    lg = io.tile([P, TC, NL], F32, tag="lg")
                    nc.sync.dma_start(out=lg, in_=lg_d[r0:r0 + P, c * TC:(c + 1) * TC, :])
                    lst = c * TC - 1 if c > 0 else 0
                    w = TC + 1 if c > 0 else TC
                    labi = labp.tile([P, TC + 1], I32, tag="labi")
                    nc.sync.dma_start(out=labi[:, :w], in_=lab_d[r0:r0 + P, lst:lst + w])
                    labf = labp.tile([P, TC + 1], F32, tag="labf")
                    nc.gpsimd.tensor_copy(out=labf[:, :w], in_=labi[:, :w])

                    msl = m_t[:, c * TC:(c + 1) * TC]
                    mof = moff[:, c * TC:(c + 1) * TC]

                    # ---- u = LSE(lgO, lgS) from raw logits ----
                    lgO = lg[:, :, 2]
                    lgS = lg[:, :, 4]
                    mx_ = tmp.tile([P, TC], F32, tag="tmp")
                    nc.vector.tensor_tensor(out=mx_, in0=lgO, in1=lgS, op=OP.max)
                    mn_ = tmp.tile([P, TC], F32, tag="tmp")
                    nc.vector.tensor_tensor(out=mn_, in0=lgO, in1=lgS, op=OP.min)
                    dd = tmp.tile([P, TC], F32, tag="tmp")
                    nc.vector.tensor_sub(out=dd, in0=mn_, in1=mx_)
                    eu = tmp.tile([P, TC], F32, tag="tmp")
                    nc.scalar.activation(out=eu, in_=dd, func=AF.Exp)
                    lu = tmp.tile([P, TC], F32, tag="tmp")
                    nc.scalar.activation(out=lu, in_=eu, func=AF.Ln, bias=1.0)
                    u = tmp.tile([P, TC], F32, tag="tmp")
                    nc.vector.tensor_add(out=u, in0=mx_, in1=lu)

                    # ---- mask logits in place: lg *= m (broadcast over NL) ----
                    mb = sub_ap(m_t, c * TC, [[1, TC], [0, NL]])
                    nc.vector.tensor_tensor(out=lg[:, :, :], in0=lg[:, :, :], in1=mb,
                                            op=OP.mult)

                    # ---- masked 2x2 planes, interleaved [t, (00,01,10,11)] ----
                    nm = nmp.tile([P, TC, 4], F32, tag="nm")
                    nc.vector.tensor_tensor(out=nm[:, :, 0], in0=u, in1=msl, op=OP.mult)
                    nc.vector.tensor_tensor(out=nm[:, :, 1], in0=lg[:, :, 0], in1=mof,
                                            op=OP.add)
                    nc.vector.tensor_tensor(out=nm[:, :, 2], in0=lg[:, :, 3], in1=mof,
                                            op=OP.add)
                    nc.vector.tensor_copy(out=nm[:, :, 3], in_=lg[:, :, 1])

                    # ---- gold: state score sum (masked logits gather) ----
                    lsl = labf[:, 1:1 + TC] if c > 0 else labf[:, 0:TC]
                    for k in range(5):
                        jk = tmp.tile([P, TC], F32, tag="tmp")
                        nc.vector.scalar_tensor_tensor(
                            out=jk, in0=lsl, scalar=float(k), in1=lg[:, :, k],
                            op0=OP.is_equal, op1=OP.mult,
                            accum_out=accst[:, c * 5 + k:c * 5 + k + 1])

                    # ---- gold: transition-pairs score ----
                    npair = w - 1
                    fr = labf[:, 0:npair]
                    to = labf[:, 1:1 + npair]
                    eq1 = tmp.tile([P, TC], F32, tag="tmp")
                    nc.vector.tensor_scalar(out=eq1[:, :npair], in0=to, scalar1=1.0,
                                            scalar2=None, op0=OP.is_equal)
                    tg2 = tmp.tile([P, TC], F32, tag="tmp")
                    nc.vector.scalar_tensor_tensor(out=tg2[:, :npair], in0=to, scalar=3.0,
                                                   in1=eq1[:, :npair],
                                                   op0=OP.is_equal, op1=OP.add)
                    w13 = tmp.tile([P, TC], F32, tag="tmp")
                    nc.vector.tensor_scalar(out=w13[:, :npair], in0=tg2[:, :npair],
                                            scalar1=-2.0, scalar2=1.0,
                                            op0=OP.mult, op1=OP.add)
                    fg = tmp.tile([P, TC], F32, tag="tmp")
                    nc.vector.tensor_scalar(out=fg[:, :npair], in0=fr, scalar1=2.0,
                                            scalar2=None, op0=OP.is_ge)
                    pr = tmp.tile([P, TC], F32, tag="tmp")
                    nc.vector.tensor_tensor(out=pr[:, :npair], in0=fg[:, :npair],
                                            in1=w13[:, :npair], op=OP.mult)
                    al = tmp.tile([P, TC], F32, tag="tmp")
                    nc.vector.tensor_add(out=al[:, :npair], in0=pr[:, :npair],
                                         in1=tg2[:, :npair])
                    mp = m_t[:, c * TC:(c + 1) * TC] if c > 0 else m_t[:, 1:TC]
                    jt = tmp.tile([P, TC], F32, tag="tmp")
                    nc.vector.scalar_tensor_tensor(
                        out=jt[:, :npair], in0=al[:, :npair], scalar=1.0, in1=mp,
                        op0=OP.mult, op1=OP.mult,
                        accum_out=acctr[:, c:c + 1])

                    # ---- in-chunk tree: 512 -> 64 matrices ----
                    cur = nm[:, :, :]
                    nmat = TC
                    for lv in range(CH_LV):
                        npairs = nmat // 2
                        if lv == CH_LV - 1:
                            dst = acc[:, c * ACC_PER_CH * 4:(c + 1) * ACC_PER_CH * 4]
                        else:
                            dst = lvl.tile([P, npairs * 4], F32, tag="lvl")
                        combine(cur, npairs, dst)
                        cur = dst
                        nmat = npairs

                # ---- global tree: 256 -> 1 ----
                cur = acc[:, :]
                nmat = ACC_M
                for lv in range(GL_LV):
                    npairs = nmat // 2
                    dst = lvl.tile([P, npairs * 4], F32, tag="glvl")
                    combine(cur, npairs, dst)
                    cur = dst
                    nmat = npairs
                Pfin = cur  # [P, 4]

                # ---- finals ----
                mx4 = tmp.tile([P, 1], F32, tag="fin")
                nc.vector.reduce_max(out=mx4, in_=Pfin, axis=AX.X)
                nmx = tmp.tile([P, 1], F32, tag="fin")
                nc.vector.tensor_scalar_mul(out=nmx, in0=mx4, scalar1=-1.0)
                e4 = tmp.tile([P, 4], F32, tag="fin4")
                nc.scalar.activation(out=e4, in_=Pfin, func=AF.Exp, bias=nmx, scale=1.0)
                s4 = tmp.tile([P, 1], F32, tag="fin")
                nc.vector.reduce_sum(out=s4, in_=e4, axis=AX.X)
                ls = tmp.tile([P, 1], F32, tag="fin")
                nc.scalar.activation(out=ls, in_=s4, func=AF.Ln)
                allv = tmp.tile([P, 1], F32, tag="fin")
                nc.vector.tensor_scalar(out=allv, in0=ls, scalar1=mx4, scalar2=lenp1,
                                        op0=OP.add, op1=OP.add)

                st_ = tmp.tile([P, 1], F32, tag="fin")
                nc.vector.reduce_sum(out=st_, in_=accst, axis=AX.X)
                at_ = tmp.tile([P, 1], F32, tag="fin")
                nc.vector.reduce_sum(out=at_, in_=acctr, axis=AX.X)
                gp = tmp.tile([P, 1], F32, tag="fin")
                nc.vector.scalar_tensor_tensor(out=gp, in0=at_, scalar=10001.0, in1=st_,
                                               op0=OP.mult, op1=OP.add)
                lp = tmp.tile([P, 1], F32, tag="fin")
                nc.vector.tensor_scalar(out=lp, in0=lf, scalar1=-10000.0, scalar2=10002.0,
                                        op0=OP.mult, op1=OP.add)
                gold = tmp.tile([P, 1], F32, tag="fin")
                nc.vector.tensor_add(out=gold, in0=gp, in1=lp)

                outv = tmp.tile([P, 1], F32, tag="fin")
                nc.vector.tensor_sub(out=outv, in0=allv, in1=gold)
                nc.sync.dma_start(out=bass.AP(tensor=out_d, offset=r0,
                                              ap=[[1, P], [1, 1]]),
                                  in_=outv)
    nc.compile()
    return nc


def _numpy_fallback(logits, transition, labels, lens):
    # generic f32 forward algorithm; only used if transition doesn't match
    # the expected BIOES pattern (never on harness inputs)
    Bn, Sn, _ = logits.shape
    lg = np.concatenate([logits, np.full((Bn, Sn, 2), -INF, np.float32)], axis=2)
    prev = np.full((Bn, 7), -INF, np.float32)
    prev[:, 5] = 0.0
    for t in range(Sn):
        sc = prev[:, :, None] + transition[None] + lg[:, t][:, None, :]
        m = sc.max(1)
        nxt = m + np.log(np.exp(sc - m[:, None, :]).sum(1))
        step = (t < lens)[:, None]
        prev = np.where(step, nxt, prev)
    fin = prev + transition[:, 6][None]
    m = fin.max(1)
    alls = m + np.log(np.exp(fin - m[:, None]).sum(1))
    ext = np.concatenate([np.full((Bn, 1), 5, np.int64), labels.astype(np.int64),
                          np.full((Bn, 1), 6, np.int64)], axis=1)
    pos = np.arange(Sn + 2)[None]
    ext = np.where(pos < (lens[:, None] + 1), ext, 6)
    tr = transition[ext[:, :-1], ext[:, 1:]]
    tr = tr * (np.arange(Sn + 1)[None] < (lens[:, None] + 1))
    stv = np.take_along_axis(lg, labels[..., None].astype(np.int64), axis=2)[..., 0]
    stv = stv * (np.arange(Sn)[None] < lens[:, None])
    return (alls - (tr.sum(1) + stv.sum(1))).astype(np.float32)


def kernel(logits, transition, labels, lens):
    logits = np.ascontiguousarray(np.asarray(logits, dtype=np.float32))
    transition = np.asarray(transition, dtype=np.float32)
    labels = np.ascontiguousarray(np.asarray(labels, dtype=np.int32))
    lens = np.ascontiguousarray(np.asarray(lens, dtype=np.int32))

    if not np.array_equal(transition, _expected_transition()):
        return _numpy_fallback(logits, transition, labels, lens)

    from concourse.bass_utils import run_bass_kernel_spmd

    if "nc" not in _CACHE:
        _CACHE["nc"] = _build()
    nc = _CACHE["nc"]

    in_maps = []
    for c in range(NCORES):
        sl = slice(c * BC, (c + 1) * BC)
        in_maps.append({
            "logits": np.ascontiguousarray(logits[sl]),
            "labels": np.ascontiguousarray(labels[sl]),
            "lens": np.ascontiguousarray(lens[sl]),
        })
    res = run_bass_kernel_spmd(nc, in_maps, core_ids=list(range(NCORES)))
    return np.concatenate([res.results[c]["out"] for c in range(NCORES)])
